# revision 30
# baseline (speedup 1.0000x reference)
"""LocalRNN (windowed GRU) Trainium2 kernel.

Problem: x (16, 2048, 128) fp32; each position t gets window x[t-7..t]
(front zero-padded); a GRU (torch gate order r|z|n) runs over the 8-token
window from h=0; only the last hidden state is kept -> (16, 2048, 128).

Sharding: pure data parallel over batch: 2 rows per core on 8 cores.

Per-core layout: [d=128 partitions, positions free].  Per core the 2 batch
rows are concatenated: padded x buffers have row stride 2056 (8 pad cols,
7 of which are the required zeros; real data at col 8), h is [128, 2*2048].
At window step k, position t reads padded col t + k + 1.

Work is processed in 1024-col chunk-pairs (4 pairs per step, kept narrow
enough that four independent dependency chains pipeline across engines).
Engine assignment per pair and step (f16 everywhere; out written f16 and
upconverted on host; tolerance is 2e-2):
  PE : ps_r = W_ihr@x_k + W_hhr@h, ps_z likewise (512-wide matmuls into
       [128,1024] 2-bank psum tiles), ps_n = W_hhn@h (double-buffered)
  ACT: r = sigmoid(ps_r + b_r), z = sigmoid(ps_z + b_z)
  DVE: t = (ps_n + b_hhn) * r       (scalar_tensor_tensor)
       u = t + px_n[k shift]        (px_n precomputed; shifted copy px_o
                                     covers odd-k alignment at fp16 2x)
  ACT: n = tanh(u + b_ihn)
  Pool(GpSimd): d = h - n, w = z * d  (offloads the DVE bottleneck)
  DVE: h' = n + w
Step 0 runs with h = 0: no h/n matmuls, t = b_hhn*r via 4x tensor_scalar,
h1 = n - z*n, and the h memset is skipped entirely.
"""

import numpy as np

B, L, D, KS = 16, 2048, 128, 8
N_CORES = 8
ROWS_PER_CORE = B // N_CORES  # 2
PAD = KS  # 8 leading pad cols per row (7 required zeros + 1 for alignment)
ROWSTRIDE = L + PAD  # 2056 (even, keeps fp16 slice parity uniform in k)
PXW = ROWS_PER_CORE * ROWSTRIDE  # 4112
HW = ROWS_PER_CORE * L  # 4096
CHUNK = 512
PAIR = 1024

_cache = {}


def _build_nc():
    import concourse.mybir as mybir
    import concourse.tile as tile
    from concourse import bacc
    from contextlib import ExitStack

    f32 = mybir.dt.float32
    f16 = mybir.dt.float16
    AF = mybir.ActivationFunctionType
    ALU = mybir.AluOpType

    nc = bacc.Bacc(
        "TRN2",
        target_bir_lowering=False,
        debug=False,
        num_devices=N_CORES,
    )
    PKW = PXW + 6 * D
    packed = nc.declare_dram_parameter("packed", [D, PKW], f16, isOutput=False)
    biases = nc.declare_dram_parameter("biases", [D, 4], f32, isOutput=False)
    out = nc.declare_dram_parameter("out", [D, HW], f16, isOutput=True)

    with ExitStack() as ctx:
        tc = ctx.enter_context(tile.TileContext(nc))
        const = ctx.enter_context(tc.tile_pool(name="const", bufs=1))
        pxpool = ctx.enter_context(tc.tile_pool(name="pxpool", bufs=1))
        hpool = ctx.enter_context(tc.tile_pool(name="hpool", bufs=1))
        tmp = ctx.enter_context(tc.tile_pool(name="tmp", bufs=4))
        psum = ctx.enter_context(tc.tile_pool(name="psum", bufs=1, space="PSUM"))

        pk_sb = const.tile([D, PKW], f16, tag="pk")
        bias_sb = const.tile([D, 4], f32, tag="bias")
        # warm the activation table while DMAs stream (no data deps)
        warm = const.tile([D, 1], f16, tag="warm")
        nc.vector.memset(warm[:], 0.0)
        nc.scalar.activation(warm[:], warm[:], AF.Sigmoid)
        # input transfers: critical pieces (r/z weights, first x block, biases)
        # first on the SP queue; the rest spread to the gpsimd queue
        nc.sync.dma_start(
            pk_sb[:, PXW : PXW + 2 * D], packed[:, PXW : PXW + 2 * D]
        )  # wih r|z
        nc.sync.dma_start(pk_sb[:, 0 : PAD + CHUNK], packed[:, 0 : PAD + CHUNK])
        nc.gpsimd.dma_start(bias_sb[:], biases[:])
        nc.sync.dma_start(
            pk_sb[:, PAD + CHUNK : PAD + PAIR], packed[:, PAD + CHUNK : PAD + PAIR]
        )
        nc.sync.dma_start(
            pk_sb[:, PAD + PAIR : ROWSTRIDE], packed[:, PAD + PAIR : ROWSTRIDE]
        )
        nc.gpsimd.dma_start(pk_sb[:, PXW + 2 * D : PKW], packed[:, PXW + 2 * D : PKW])
        ro1 = ROWSTRIDE
        nc.sync.dma_start(
            pk_sb[:, ro1 : ro1 + PAD + PAIR], packed[:, ro1 : ro1 + PAD + PAIR]
        )
        nc.sync.dma_start(
            pk_sb[:, ro1 + PAD + PAIR : ro1 + ROWSTRIDE],
            packed[:, ro1 + PAD + PAIR : ro1 + ROWSTRIDE],
        )
        x_sb = pk_sb[:, 0:PXW]
        wih_sb = pk_sb[:, PXW : PXW + 3 * D]
        whh_sb = pk_sb[:, PXW + 3 * D : PXW + 6 * D]

        h_a = hpool.tile([D, HW], f16, tag="h_a")
        h_b = hpool.tile([D, HW], f16, tag="h_b")
        px_e = pxpool.tile([D, PXW], f16, tag="px_e", name="px_e")
        px_o = pxpool.tile([D, PXW], f16, tag="px_o", name="px_o")

        prefetched = {}  # (k, p) -> (ps_r, ps_z) with x-matmuls already emitted

        def gates_x(k, p):
            """Allocate ps_r/ps_z for (k, p) and emit the W_ih@x matmuls
            (no h dependency, so they can fill PE across step boundaries)."""
            row = p // (L // PAIR)
            po = row * ROWSTRIDE + (k + 1) + (p % (L // PAIR)) * PAIR
            ps_r = psum.tile([D, PAIR], f32, tag="ps_r", name="ps_r")
            ps_z = psum.tile([D, PAIR], f32, tag="ps_z", name="ps_z")
            for half in range(2):
                o = half * CHUNK
                xs = x_sb[:, po + o : po + o + CHUNK]
                nc.tensor.matmul(
                    ps_r[:, o : o + CHUNK], wih_sb[:, 0:D], xs,
                    start=True, stop=(k == 0),
                )
                nc.tensor.matmul(
                    ps_z[:, o : o + CHUNK], wih_sb[:, D : 2 * D], xs,
                    start=True, stop=(k == 0),
                )
            return ps_r, ps_z

        def gates(k, p, h_src):
            """Emit matmuls + sigmoids + t for step k, pair p. Returns (r,z,t)."""
            ho = p * PAIR
            if (k, p) in prefetched:
                ps_r, ps_z = prefetched.pop((k, p))
            else:
                ps_r, ps_z = gates_x(k, p)
            if k > 0:
                for half in range(2):
                    o = half * CHUNK
                    hh = h_src[:, ho + o : ho + o + CHUNK]
                    nc.tensor.matmul(
                        ps_r[:, o : o + CHUNK], whh_sb[:, 0:D], hh,
                        start=False, stop=True,
                    )
                    nc.tensor.matmul(
                        ps_z[:, o : o + CHUNK], whh_sb[:, D : 2 * D], hh,
                        start=False, stop=True,
                    )
            r = tmp.tile([D, PAIR], f16, tag="r", name="r")
            nc.scalar.activation(r[:], ps_r[:], AF.Sigmoid, bias=bias_sb[:, 0:1])
            z = tmp.tile([D, PAIR], f16, tag="z", name="z")
            nc.scalar.activation(z[:], ps_z[:], AF.Sigmoid, bias=bias_sb[:, 1:2])

            t = tmp.tile([D, PAIR], f16, tag="t", name="t")
            if k == 0:
                # h = 0: gh_n = 0, so t = b_hhn * r (tensor_scalar, 4x)
                nc.vector.tensor_scalar_mul(t[:], r[:], bias_sb[:, 3:4])
            else:
                ps_n = psum.tile([D, PAIR], f32, tag="ps_n", name="ps_n", bufs=2)
                for half in range(2):
                    o = half * CHUNK
                    nc.tensor.matmul(
                        ps_n[:, o : o + CHUNK], whh_sb[:, 2 * D : 3 * D],
                        h_src[:, ho + o : ho + o + CHUNK],
                        start=True, stop=True,
                    )
                # t = (ps_n + b_hhn) * r
                nc.vector.scalar_tensor_tensor(
                    t[:], ps_n[:], bias_sb[:, 3:4], r[:], op0=ALU.add, op1=ALU.mult
                )
            return r, z, t

        def pxn_slice(k, p):
            row = p // (L // PAIR)
            po = row * ROWSTRIDE + (k + 1) + (p % (L // PAIR)) * PAIR
            if po % 2 == 0:
                return px_e[:, po : po + PAIR]
            return px_o[:, po - 1 : po - 1 + PAIR]

        def update(k, p, z, n_ap, h_src, h_dst):
            """h' = n + z*(h - n) for step k, pair p (n_ap: [D, PAIR] view)."""
            ho = p * PAIR
            hs = h_src[:, ho : ho + PAIR]
            d = tmp.tile([D, PAIR], f16, tag="d", name="d")
            nc.gpsimd.tensor_tensor(d[:], hs, n_ap, op=ALU.subtract)
            w_ = tmp.tile([D, PAIR], f16, tag="w", name="w")
            nc.gpsimd.tensor_tensor(w_[:], z[:], d[:], op=ALU.mult)
            if k == KS - 1:
                hd = tmp.tile([D, PAIR], f16, tag="hf", name="hf")
                nc.vector.tensor_add(hd[:], n_ap, w_[:])
                nc.sync.dma_start(out[:, ho : ho + PAIR], hd[:])
            else:
                nc.vector.tensor_add(h_dst[:, ho : ho + PAIR], n_ap, w_[:])

        # --- step 0 gate phase (no px dependency) ---
        step0 = [gates(0, p, None) for p in range(HW // PAIR)]

        # --- precompute n-gate input projection px_n = W_ihn @ x ---
        # (runs while ACT chews on the step-0 sigmoids)
        for row in range(ROWS_PER_CORE):
            ro = row * ROWSTRIDE
            for o, w in ((0, PAIR), (PAIR, PAIR), (2 * PAIR, ROWSTRIDE - 2 * PAIR)):
                ps = psum.tile([D, PAIR], f32, tag="ps_n", name="ps_px", bufs=2)
                for cc in range(0, w, CHUNK):
                    cw = min(CHUNK, w - cc)
                    nc.tensor.matmul(
                        ps[:, cc : cc + cw],
                        wih_sb[:, 2 * D : 3 * D],
                        x_sb[:, ro + o + cc : ro + o + cc + cw],
                        start=True,
                        stop=True,
                    )
                nc.vector.tensor_copy(px_e[:, ro + o : ro + o + w], ps[:, :w])
            # shifted copy for odd-k alignment: px_o[:, j] = px_e[:, j+1]
            nc.vector.tensor_copy(
                px_o[:, ro : ro + ROWSTRIDE - 1], px_e[:, ro + 1 : ro + ROWSTRIDE]
            )

        # keep PE warm across the step-0 -> step-1 boundary
        prefetched[(1, 0)] = gates_x(1, 0)

        # --- step 0 tail phase: h1 = n - z*n ---
        for p, (r, z, t) in enumerate(step0):
            ho = p * PAIR
            u = tmp.tile([D, PAIR], f16, tag="u", name="u")
            nc.vector.tensor_add(u[:], t[:], pxn_slice(0, p))
            n = tmp.tile([D, PAIR], f16, tag="n", name="n")
            nc.scalar.activation(n[:], u[:], AF.Tanh, bias=bias_sb[:, 2:3])
            w_ = tmp.tile([D, PAIR], f16, tag="w", name="w")
            nc.gpsimd.tensor_tensor(w_[:], z[:], n[:], op=ALU.mult)
            nc.vector.tensor_sub(h_b[:, ho : ho + PAIR], n[:], w_[:])

        # --- steps 1..6 ---
        for k in range(1, KS - 1):
            h_src, h_dst = (h_a, h_b) if k % 2 == 0 else (h_b, h_a)
            for p in range(HW // PAIR):
                r, z, t = gates(k, p, h_src)
                u = tmp.tile([D, PAIR], f16, tag="u", name="u")
                nc.vector.tensor_add(u[:], t[:], pxn_slice(k, p))
                n = tmp.tile([D, PAIR], f16, tag="n", name="n")
                nc.scalar.activation(n[:], u[:], AF.Tanh, bias=bias_sb[:, 2:3])
                update(k, p, z, n[:], h_src, h_dst)
            # keep PE warm across the step boundary
            prefetched[(k + 1, 0)] = gates_x(k + 1, 0)

        # --- step 7: write out; final pair drains through narrow 512 chains ---
        k = KS - 1
        h_src = h_a if k % 2 == 0 else h_b
        for p in range(HW // PAIR):
            r, z, t = gates(k, p, h_src)
            ho = p * PAIR
            if p < HW // PAIR - 1:
                u = tmp.tile([D, PAIR], f16, tag="u", name="u")
                nc.vector.tensor_add(u[:], t[:], pxn_slice(k, p))
                n = tmp.tile([D, PAIR], f16, tag="n", name="n")
                nc.scalar.activation(n[:], u[:], AF.Tanh, bias=bias_sb[:, 2:3])
                update(k, p, z, n[:], h_src, None)
            else:
                pxn = pxn_slice(k, p)
                hs = h_src[:, ho : ho + PAIR]
                for cc, cw in ((0, CHUNK), (CHUNK, CHUNK)):
                    uc = tmp.tile([D, CHUNK], f16, tag="uc", name="uc", bufs=2)
                    nc.vector.tensor_add(uc[:, :cw], t[:, cc : cc + cw],
                                         pxn[:, cc : cc + cw])
                    nn = tmp.tile([D, CHUNK], f16, tag="nc", name="nn", bufs=2)
                    nc.scalar.activation(nn[:, :cw], uc[:, :cw], AF.Tanh,
                                         bias=bias_sb[:, 2:3])
                    dc = tmp.tile([D, CHUNK], f16, tag="dc", name="dc", bufs=2)
                    nc.vector.tensor_sub(dc[:, :cw], hs[:, cc : cc + cw], nn[:, :cw])
                    wc = tmp.tile([D, CHUNK], f16, tag="wc", name="wc", bufs=2)
                    nc.vector.tensor_mul(wc[:, :cw], z[:, cc : cc + cw], dc[:, :cw])
                    hc = tmp.tile([D, CHUNK], f16, tag="hc", name="hc", bufs=2)
                    nc.vector.tensor_add(hc[:, :cw], nn[:, :cw], wc[:, :cw])
                    nc.sync.dma_start(out[:, ho + cc : ho + cc + cw], hc[:, :cw])
    nc.compile()
    return nc


def _get_nc():
    if "nc" not in _cache:
        _cache["nc"] = _build_nc()
    return _cache["nc"]


def _prep_in_maps(x, W_ih, W_hh, b_ih, b_hh):
    x = np.asarray(x, dtype=np.float32)
    assert x.shape == (B, L, D)
    W_ih = np.asarray(W_ih, np.float32)
    W_hh = np.asarray(W_hh, np.float32)
    b_ih = np.asarray(b_ih, np.float32)
    b_hh = np.asarray(b_hh, np.float32)

    wihT = W_ih.T.astype(np.float16)  # [d, 3d]
    whhT = W_hh.T.astype(np.float16)
    biases = np.stack(
        [
            b_ih[:D] + b_hh[:D],  # sigmoid bias r
            b_ih[D : 2 * D] + b_hh[D : 2 * D],  # sigmoid bias z
            b_ih[2 * D :],  # tanh bias (b_ihn)
            b_hh[2 * D :],  # stt scalar: ps_n + b_hhn
        ],
        axis=1,
    ).astype(np.float32)  # [128, 4]

    PKW = PXW + 6 * D
    in_maps = []
    for c in range(N_CORES):
        pk = np.zeros((D, PKW), np.float16)
        for r in range(ROWS_PER_CORE):
            row = x[c * ROWS_PER_CORE + r]  # (L, D)
            pk[:, r * ROWSTRIDE + PAD : (r + 1) * ROWSTRIDE] = row.T.astype(np.float16)
        pk[:, PXW : PXW + 3 * D] = wihT
        pk[:, PXW + 3 * D : PXW + 6 * D] = whhT
        in_maps.append({"packed": pk, "biases": biases})
    return in_maps


def kernel(x, W_ih, W_hh, b_ih, b_hh, ksize):
    from concourse.bass_utils import run_bass_kernel_spmd

    assert int(ksize) == KS
    in_maps = _prep_in_maps(x, W_ih, W_hh, b_ih, b_hh)
    nc = _get_nc()
    results = run_bass_kernel_spmd(nc, in_maps, list(range(N_CORES))).results

    y = np.empty((B, L, D), np.float32)
    for c in range(N_CORES):
        o = results[c]["out"]  # [D, HW] f16
        for r in range(ROWS_PER_CORE):
            y[c * ROWS_PER_CORE + r] = o[:, r * L : (r + 1) * L].T.astype(np.float32)
    return y
